# revision 6
# baseline (speedup 1.0000x reference)
"""Cross-attention (B=16, S=2048, D=1024, fp32) on 8 TRN2 NeuronCores.

Sharding: data-parallel over batch (2 batches per core), weights replicated.

All five matmuls run in fp8e4 (TRN FP8_EXP4, max +-240) with
perf_mode=DoubleRow: operands are [128, 2, N] k-pair tiles, the PE
virtualizes to a 128x256 array and contracts 256 elements per instruction
at 0.5 cycles/row -- ~2x bf16 FLOP rate and half the LDWEIGHTS count of the
fp32r baseline (whose 4-byte weights also forbid background weight-buffer
overlap, serializing every weight load).

Scaling scheme (validated on host against the jax reference, rel err 4.5e-3
vs the 2e-2 gate):
  host:   x8=fp8(x), y8=fp8(y), W8=fp8(32*W)      (32*W ~ N(0,1) fills the
                                                    fp8 range; bare W would
                                                    be mostly subnormal)
  device: Q8 = fp8(psum/32 + bq)   K8 likewise    (ACT / DVE descale+bias)
          V8 = fp8(psum/32 + bv)
          logitsT[k,q] = K8^T Q8                   (fp8 DR matmul, fp32 psum)
          e8[k,q] = fp8(exp(logits/sqrt(D) - 2))   (-2 bias keeps exp <= ~60,
                                                    under fp8e4's 240 max;
                                                    softmax is shift-invariant)
          Z[q]   = ones^T e8        ([1,512] psum row, ones stationary)
          out_u  = e8^T V8          (per 128-wide q chunk, both 512-f halves)
          out    = out_u * (1/(Z+eps)) + x         (fp32 residual from DRAM)

Z's [1,512] row is transposed to [128,4] per-partition scalars by a tiny
4-byte-element SBUF->SBUF scatter DMA (off the compute engines).

Everything is SBUF-resident in fp8 (Q/K/V 2MB each per batch) -- no DRAM
spill, y loaded once for both K and V. fp8 inputs ship as uint8 DRAM
tensors (bitcast on device) to keep jax/PJRT out of fp8 dtypes.
"""

import numpy as np
from contextlib import ExitStack

import concourse.bacc as bacc
import concourse.tile as tile
import concourse.mybir as mybir
from concourse.bass_utils import run_bass_kernel_spmd

# problem dims (hardcoded per harness contract)
B, S, D = 16, 2048, 1024
NCORES, P = 8, 128
BPC = B // NCORES          # 2 batches per core
NFC = D // P               # 8 feature chunks of 128
NDC = D // P               # 8 contraction chunks of 128
NKT = S // P               # 16 key chunks of 128
W5 = 512
NST = S // W5              # 4 strips of 512
NDH = D // W5              # 2 output-feature halves of 512
NPAIR = NKT // 2           # 8 key-chunk pairs (DoubleRow granularity)
WSCALE = 32.0              # host premultiplies weights by this
SM_SCALE = float(1.0 / np.sqrt(D))
EXP_BIAS = -2.0
EPS = 1e-6

F32 = mybir.dt.float32
F8 = mybir.dt.float8e4
DR = mybir.MatmulPerfMode.DoubleRow

AF = mybir.ActivationFunctionType
ALU = mybir.AluOpType


def _build():
    nc = bacc.Bacc("TRN2", target_bir_lowering=False, debug=False)

    x8T = nc.dram_tensor("x8T", [BPC, D, S], mybir.dt.uint8, kind="ExternalInput").ap().bitcast(F8)
    y8T = nc.dram_tensor("y8T", [BPC, D, S], mybir.dt.uint8, kind="ExternalInput").ap().bitcast(F8)
    xr = nc.dram_tensor("xr", [BPC, S, D], F32, kind="ExternalInput").ap()
    w8q = nc.dram_tensor("w8q", [D, D], mybir.dt.uint8, kind="ExternalInput").ap().bitcast(F8)
    w8k = nc.dram_tensor("w8k", [D, D], mybir.dt.uint8, kind="ExternalInput").ap().bitcast(F8)
    w8v = nc.dram_tensor("w8v", [D, D], mybir.dt.uint8, kind="ExternalInput").ap().bitcast(F8)
    bq = nc.dram_tensor("bq", [D], F32, kind="ExternalInput").ap()
    bk = nc.dram_tensor("bk", [D], F32, kind="ExternalInput").ap()
    bv = nc.dram_tensor("bv", [D], F32, kind="ExternalInput").ap()
    out = nc.dram_tensor("out", [BPC, S, D], F32, kind="ExternalOutput").ap()

    RSC = float(1.0 / WSCALE)

    with tile.TileContext(nc) as tc, ExitStack() as ctx:
        const = ctx.enter_context(tc.tile_pool(name="const", bufs=1))
        kvp = ctx.enter_context(tc.tile_pool(name="kvp", bufs=1))
        expp = ctx.enter_context(tc.tile_pool(name="expp", bufs=1))
        psum = ctx.enter_context(tc.tile_pool(name="psum", bufs=4, space="PSUM"))
        dram = ctx.enter_context(tc.tile_pool(name="dram", bufs=2, space="DRAM"))

        # ---- constants
        onesf = const.tile([P, 2, 16], F32)
        nc.vector.memset(onesf, 1.0)
        ones8 = const.tile([P, 2, 16], F8)
        nc.vector.tensor_copy(ones8, onesf)
        bqs = const.tile([P, NFC], F32)
        nc.gpsimd.dma_start(out=bqs, in_=bq.rearrange("(fc p) -> p fc", p=P))
        bks = const.tile([P, NFC], F32)
        nc.gpsimd.dma_start(out=bks, in_=bk.rearrange("(fc p) -> p fc", p=P))
        import concourse.bass as bass
        bvb = const.tile([P, D], F32)
        bv1 = bv.rearrange("(a d) -> a d", a=1)
        bv_bcast = bass.AP(tensor=bv1.tensor, offset=bv1.offset,
                           ap=[[0, P]] + list(bv1.ap[1:]))
        nc.gpsimd.dma_start(out=bvb, in_=bv_bcast)
        ebias = const.tile([P, 1], F32)
        nc.vector.memset(ebias, EXP_BIAS)

        # ---- projection weights, resident fp8 [P, dc, f]
        wq = const.tile([P, NDC, D], F8, name="wq")
        nc.sync.dma_start(out=wq, in_=w8q.rearrange("(dc p) f -> p dc f", p=P))
        wk = const.tile([P, NDC, D], F8, name="wk")
        nc.sync.dma_start(out=wk, in_=w8k.rearrange("(dc p) f -> p dc f", p=P))
        wv = const.tile([P, NDC, D], F8, name="wv")
        nc.sync.dma_start(out=wv, in_=w8v.rearrange("(dc p) f -> p dc f", p=P))

        for b in range(BPC):
            # resident fp8 operands for this batch
            QT = kvp.tile([P, NFC, S], F8, tag="QT")      # f-major
            KT = kvp.tile([P, NFC, S], F8, tag="KT")      # f-major
            V = kvp.tile([P, NKT, D], F8, tag="V")        # seq-major

            # ================= stage A: projections =================
            with tc.tile_pool(name=f"stA_{b}", bufs=2) as strips:
                for st in range(NST):
                    sl = slice(st * W5, (st + 1) * W5)
                    xq = strips.tile([P, NDC, W5], F8, tag="xq", name="xq")
                    nc.sync.dma_start(out=xq, in_=x8T[b, :, sl].rearrange("(dc p) s -> p dc s", p=P))
                    yq = strips.tile([P, NDC, W5], F8, tag="yq", name="yq")
                    nc.sync.dma_start(out=yq, in_=y8T[b, :, sl].rearrange("(dc p) s -> p dc s", p=P))

                    # Q: f-major, ACT descale+bias -> fp8
                    for fc in range(NFC):
                        tg = "ao" if fc % 2 == 0 else "lg"
                        ps = psum.tile([P, W5], F32, tag=tg,
                                       bufs=(4 if tg == "ao" else 3), name="psq")
                        for dp in range(NDC // 2):
                            nc.tensor.matmul(ps, wq[:, 2 * dp:2 * dp + 2, fc * P:(fc + 1) * P],
                                             xq[:, 2 * dp:2 * dp + 2, :],
                                             start=(dp == 0), stop=(dp == NDC // 2 - 1),
                                             perf_mode=DR)
                        nc.scalar.activation(QT[:, fc, sl], ps, AF.Identity,
                                             bias=bqs[:, fc:fc + 1], scale=RSC)

                    # K: f-major, DVE descale+bias -> fp8
                    for fc in range(NFC):
                        tg = "ao" if fc % 2 == 0 else "lg"
                        ps = psum.tile([P, W5], F32, tag=tg,
                                       bufs=(4 if tg == "ao" else 3), name="psk")
                        for dp in range(NDC // 2):
                            nc.tensor.matmul(ps, wk[:, 2 * dp:2 * dp + 2, fc * P:(fc + 1) * P],
                                             yq[:, 2 * dp:2 * dp + 2, :],
                                             start=(dp == 0), stop=(dp == NDC // 2 - 1),
                                             perf_mode=DR)
                        nc.vector.tensor_scalar(KT[:, fc, sl], ps, RSC, bks[:, fc:fc + 1],
                                                op0=ALU.mult, op1=ALU.add)

                    # V: seq-major, DVE descale+bias -> fp8
                    for ks in range(NST):
                        kt = st * NST + ks
                        for dh in range(NDH):
                            tg = "ao" if (ks + dh) % 2 == 0 else "lg"
                            ps = psum.tile([P, W5], F32, tag=tg,
                                           bufs=(4 if tg == "ao" else 3), name="psv")
                            for dp in range(NDC // 2):
                                nc.tensor.matmul(ps, yq[:, 2 * dp:2 * dp + 2, ks * P:(ks + 1) * P],
                                                 wv[:, 2 * dp:2 * dp + 2, dh * W5:(dh + 1) * W5],
                                                 start=(dp == 0), stop=(dp == NDC // 2 - 1),
                                                 perf_mode=DR)
                            nc.vector.scalar_tensor_tensor(
                                V[:, kt, dh * W5:(dh + 1) * W5], ps, RSC,
                                bvb[:, dh * W5:(dh + 1) * W5],
                                op0=ALU.mult, op1=ALU.add)

            # ================= stage B: attention =================
            # Per 512-wide q strip: logitsT[k,q] in psum -> exp to fp8 SBUF
            # (16 [128,512] k-chunk tiles); Z accumulated as a [1,512] psum row
            # (ones stationary); then attn@V per 128-wide q chunk with the exp
            # tiles replayed as DoubleRow stationary operands.
            with tc.tile_pool(name=f"stB_{b}", bufs=4) as bp:
                for st in range(NST):
                    sl = slice(st * W5, (st + 1) * W5)
                    exs = expp.tile([P, NKT, W5], F8, tag="exT", bufs=2)
                    xrs = bp.tile([P, NST, D], F32, tag="xrs", bufs=2, name="xrs")
                    nc.sync.dma_start(
                        out=xrs, in_=xr[b, sl, :].rearrange("(c p) d -> p c d", p=P))
                    zps = psum.tile([P, W5], F32, tag="z", bufs=1, name="zps")

                    def z_acc(pr):
                        nc.tensor.matmul(zps[0:1, :], ones8[:, :, 0:1],
                                         exs[:, 2 * pr:2 * pr + 2, :],
                                         start=(pr == 0), stop=(pr == NPAIR - 1),
                                         perf_mode=DR)

                    for kc in range(NKT):
                        lg = psum.tile([P, W5], F32, tag="lg", bufs=3, name="lg")
                        for fp in range(NFC // 2):
                            nc.tensor.matmul(lg, KT[:, 2 * fp:2 * fp + 2, kc * P:(kc + 1) * P],
                                             QT[:, 2 * fp:2 * fp + 2, sl],
                                             start=(fp == 0), stop=(fp == NFC // 2 - 1),
                                             perf_mode=DR)
                        nc.scalar.activation(exs[:, kc, :], lg, AF.Exp,
                                             bias=ebias[:, 0:1], scale=SM_SCALE)
                        # z pair (kc-1)/2 one pair late so exp is long done
                        if kc % 2 == 1 and kc > 1:
                            z_acc((kc - 1) // 2 - 1)

                    # attn@V, one 128-wide q chunk at a time; first chunk's
                    # matmuls hide the final z pair + Z->rz transpose latency
                    rzc = bp.tile([P, NST], F32, tag="rzc", name="rzc")
                    for qq in range(NST):
                        qsl = slice(qq * P, (qq + 1) * P)
                        aos = [psum.tile([P, W5], F32, tag="ao", name=f"ao{dh}")
                               for dh in range(NDH)]
                        for pr in range(NPAIR):
                            for dh in range(NDH):
                                nc.tensor.matmul(aos[dh], exs[:, 2 * pr:2 * pr + 2, qsl],
                                                 V[:, 2 * pr:2 * pr + 2, dh * W5:(dh + 1) * W5],
                                                 start=(pr == 0), stop=(pr == NPAIR - 1),
                                                 perf_mode=DR)
                            if qq == 0 and pr == 0:
                                z_acc(NPAIR - 1)
                                # Z row -> [128,4] per-partition scalars via a
                                # DRAM bounce (SBUF APs can't express the
                                # transposing view) -> 1/(Z+eps)
                                z2 = bp.tile([P, W5], F32, tag="z2", bufs=1, name="z2")
                                nc.vector.tensor_scalar_add(z2[0:1, :], zps[0:1, :], EPS)
                                zdr = dram.tile([1, W5], F32, tag="zdr")
                                nc.sync.dma_start(out=zdr, in_=z2[0:1, :])
                                zc = bp.tile([P, NST], F32, tag="zc", name="zc")
                                nc.sync.dma_start(
                                    out=zc,
                                    in_=zdr.rearrange("a (q p) -> (a p) q", p=P))
                                nc.vector.reciprocal(rzc, zc)
                        for dh in range(NDH):
                            ob = bp.tile([P, W5], F32, tag="osb", name="ob")
                            nc.vector.scalar_tensor_tensor(
                                ob, aos[dh], rzc[:, qq:qq + 1],
                                xrs[:, qq, dh * W5:(dh + 1) * W5],
                                op0=ALU.mult, op1=ALU.add)
                            nc.sync.dma_start(
                                out=out[b, st * W5 + qq * P:st * W5 + (qq + 1) * P,
                                        dh * W5:(dh + 1) * W5],
                                in_=ob)

    nc.compile()
    return nc


_NC_CACHE = {}


def _get_nc():
    if "nc" not in _NC_CACHE:
        _NC_CACHE["nc"] = _build()
    return _NC_CACHE["nc"]


def _make_in_maps(x, y, Wq, bq, Wk, bk, Wv, bv):
    f8 = mybir.dt.np(F8)
    x = np.asarray(x, dtype=np.float32)
    y = np.asarray(y, dtype=np.float32)
    x8T = np.ascontiguousarray(x.transpose(0, 2, 1)).astype(f8).view(np.uint8)
    y8T = np.ascontiguousarray(y.transpose(0, 2, 1)).astype(f8).view(np.uint8)
    w8q = (np.asarray(Wq, dtype=np.float32) * WSCALE).astype(f8).view(np.uint8)
    w8k = (np.asarray(Wk, dtype=np.float32) * WSCALE).astype(f8).view(np.uint8)
    w8v = (np.asarray(Wv, dtype=np.float32) * WSCALE).astype(f8).view(np.uint8)
    bq = np.ascontiguousarray(np.asarray(bq, dtype=np.float32))
    bk = np.ascontiguousarray(np.asarray(bk, dtype=np.float32))
    bv = np.ascontiguousarray(np.asarray(bv, dtype=np.float32))
    in_maps = []
    for c in range(NCORES):
        sl = slice(c * BPC, (c + 1) * BPC)
        in_maps.append({
            "x8T": np.ascontiguousarray(x8T[sl]),
            "y8T": np.ascontiguousarray(y8T[sl]),
            "xr": np.ascontiguousarray(x[sl]),
            "w8q": w8q, "w8k": w8k, "w8v": w8v,
            "bq": bq, "bk": bk, "bv": bv,
        })
    return in_maps


def kernel(x, y, Wq, bq, Wk, bk, Wv, bv):
    nc = _get_nc()
    in_maps = _make_in_maps(x, y, Wq, bq, Wk, bk, Wv, bv)
    res = run_bass_kernel_spmd(nc, in_maps, core_ids=list(range(NCORES)))
    return np.concatenate([r["out"] for r in res.results], axis=0)


# revision 11
# speedup vs baseline: 1.6259x; 1.6259x over previous
"""Cross-attention (B=16, S=2048, D=1024, fp32) on 8 TRN2 NeuronCores.

Sharding: data-parallel over batch (2 batches per core), weights replicated.

All five matmuls run in fp8e4 (TRN FP8_EXP4, max +-240) with
perf_mode=DoubleRow: operands are [128, 2, N] k-pair tiles, the PE
virtualizes to a 128x256 array and contracts 256 elements per instruction
at 0.5 cycles/row -- ~2x bf16 FLOP rate and half the LDWEIGHTS count of the
fp32r baseline (whose 4-byte weights also forbid background weight-buffer
overlap, serializing every weight load).

Scaling scheme (validated on host against the jax reference, rel err 4.5e-3
vs the 2e-2 gate):
  host:   x8=fp8(x), y8=fp8(y), W8=fp8(32*W)      (32*W ~ N(0,1) fills the
                                                    fp8 range; bare W would
                                                    be mostly subnormal)
  device: Q8 = fp8(psum/32 + bq)   K8 likewise    (ACT / DVE descale+bias)
          V8 = fp8(psum/32 + bv)
          logitsT[k,q] = K8^T Q8                   (fp8 DR matmul, fp32 psum)
          e8[k,q] = fp8(exp(logits/sqrt(D) - 2))   (-2 bias keeps exp <= ~60,
                                                    under fp8e4's 240 max;
                                                    softmax is shift-invariant)
          Z[q]   = ones^T e8        ([1,512] psum row, ones stationary)
          out_u  = e8^T V8          (per 128-wide q chunk, both 512-f halves)
          out    = out_u * (1/(Z+eps)) + x         (fp32 residual from DRAM)

Z's [1,512] row is transposed to [128,4] per-partition scalars by a tiny
4-byte-element SBUF->SBUF scatter DMA (off the compute engines).

Everything is SBUF-resident in fp8 (Q/K/V 2MB each per batch) -- no DRAM
spill, y loaded once for both K and V. fp8 inputs ship as uint8 DRAM
tensors (bitcast on device) to keep jax/PJRT out of fp8 dtypes.
"""

import os
import numpy as np
from contextlib import ExitStack

# section-bisect switch for perf diagnosis: full | proj | noattn | noz
KBENCH = os.environ.get("KBENCH", "full")

import concourse.bacc as bacc
import concourse.tile as tile
import concourse.mybir as mybir
from concourse.bass_utils import run_bass_kernel_spmd

# problem dims (hardcoded per harness contract)
B, S, D = 16, 2048, 1024
NCORES, P = 8, 128
BPC = B // NCORES          # 2 batches per core
NFC = D // P               # 8 feature chunks of 128
NDC = D // P               # 8 contraction chunks of 128
NKT = S // P               # 16 key chunks of 128
W5 = 512
NST = S // W5              # 4 strips of 512
NDH = D // W5              # 2 output-feature halves of 512
NPAIR = NKT // 2           # 8 key-chunk pairs (DoubleRow granularity)
WSCALE = 32.0              # host premultiplies weights by this
SM_SCALE = float(1.0 / np.sqrt(D))
EXP_BIAS = -2.0
EPS = 1e-6

F32 = mybir.dt.float32
F8 = mybir.dt.float8e4
DR = mybir.MatmulPerfMode.DoubleRow

AF = mybir.ActivationFunctionType
ALU = mybir.AluOpType


def _build():
    nc = bacc.Bacc("TRN2", target_bir_lowering=False, debug=False)

    x8T = nc.dram_tensor("x8T", [BPC, D, S], mybir.dt.uint8, kind="ExternalInput").ap().bitcast(F8)
    y8T = nc.dram_tensor("y8T", [BPC, D, S], mybir.dt.uint8, kind="ExternalInput").ap().bitcast(F8)
    xr = nc.dram_tensor("xr", [BPC, S, D], F32, kind="ExternalInput").ap()
    w8q = nc.dram_tensor("w8q", [D, D], mybir.dt.uint8, kind="ExternalInput").ap().bitcast(F8)
    w8k = nc.dram_tensor("w8k", [D, D], mybir.dt.uint8, kind="ExternalInput").ap().bitcast(F8)
    w8v = nc.dram_tensor("w8v", [D, D], mybir.dt.uint8, kind="ExternalInput").ap().bitcast(F8)
    bq = nc.dram_tensor("bq", [D], F32, kind="ExternalInput").ap()
    bk = nc.dram_tensor("bk", [D], F32, kind="ExternalInput").ap()
    bv = nc.dram_tensor("bv", [D], F32, kind="ExternalInput").ap()
    out = nc.dram_tensor("out", [BPC, S, D], F32, kind="ExternalOutput").ap()

    RSC = float(1.0 / WSCALE)

    with tile.TileContext(nc) as tc, ExitStack() as ctx:
        const = ctx.enter_context(tc.tile_pool(name="const", bufs=1))
        kvp = ctx.enter_context(tc.tile_pool(name="kvp", bufs=1))
        expp = ctx.enter_context(tc.tile_pool(name="expp", bufs=1))
        psum = ctx.enter_context(tc.tile_pool(name="psum", bufs=4, space="PSUM"))
        dram = ctx.enter_context(tc.tile_pool(name="dram", bufs=2, space="DRAM"))

        # ---- constants
        onesf = const.tile([P, 2, 16], F32)
        nc.vector.memset(onesf, 1.0)
        ones8 = const.tile([P, 2, 16], F8)
        nc.vector.tensor_copy(ones8, onesf)
        bqs = const.tile([P, NFC], F32)
        nc.gpsimd.dma_start(out=bqs, in_=bq.rearrange("(fc p) -> p fc", p=P))
        bks = const.tile([P, NFC], F32)
        nc.gpsimd.dma_start(out=bks, in_=bk.rearrange("(fc p) -> p fc", p=P))
        import concourse.bass as bass
        bvb = const.tile([P, D], F32)
        bv1 = bv.rearrange("(a d) -> a d", a=1)
        bv_bcast = bass.AP(tensor=bv1.tensor, offset=bv1.offset,
                           ap=[[0, P]] + list(bv1.ap[1:]))
        nc.gpsimd.dma_start(out=bvb, in_=bv_bcast)
        ebias = const.tile([P, 1], F32)
        nc.vector.memset(ebias, EXP_BIAS)

        # ---- projection weights, resident fp8 [P, dc, f]
        wq = const.tile([P, NDC, D], F8, name="wq")
        nc.sync.dma_start(out=wq, in_=w8q.rearrange("(dc p) f -> p dc f", p=P))
        wk = const.tile([P, NDC, D], F8, name="wk")
        nc.sync.dma_start(out=wk, in_=w8k.rearrange("(dc p) f -> p dc f", p=P))
        wv = const.tile([P, NDC, D], F8, name="wv")
        nc.sync.dma_start(out=wv, in_=w8v.rearrange("(dc p) f -> p dc f", p=P))

        for b in range(BPC):
            # resident fp8 operands for this batch
            QT = kvp.tile([P, NFC, S], F8, tag="QT")      # f-major
            KT = kvp.tile([P, NFC, S], F8, tag="KT")      # f-major
            V = kvp.tile([P, NKT, D], F8, tag="V")        # seq-major

            # ================= stage A: projections =================
            with tc.tile_pool(name=f"stA_{b}", bufs=2) as strips:
                for st in range(NST):
                    sl = slice(st * W5, (st + 1) * W5)
                    xq = strips.tile([P, NDC, W5], F8, tag="xq", name="xq")
                    nc.sync.dma_start(out=xq, in_=x8T[b, :, sl].rearrange("(dc p) s -> p dc s", p=P))
                    yq = strips.tile([P, NDC, W5], F8, tag="yq", name="yq")
                    nc.sync.dma_start(out=yq, in_=y8T[b, :, sl].rearrange("(dc p) s -> p dc s", p=P))

                    # Q: f-major, ACT descale+bias -> fp8
                    for fc in range(NFC):
                        tg = "ao" if fc % 2 == 0 else "lg"
                        ps = psum.tile([P, W5], F32, tag=tg,
                                       bufs=(4 if tg == "ao" else 3), name="psq")
                        for dp in range(NDC // 2):
                            nc.tensor.matmul(ps, wq[:, 2 * dp:2 * dp + 2, fc * P:(fc + 1) * P],
                                             xq[:, 2 * dp:2 * dp + 2, :],
                                             start=(dp == 0), stop=(dp == NDC // 2 - 1),
                                             perf_mode=DR)
                        nc.scalar.activation(QT[:, fc, sl], ps, AF.Identity,
                                             bias=bqs[:, fc:fc + 1], scale=RSC)

                    # K: f-major, DVE descale+bias -> fp8
                    for fc in range(NFC):
                        tg = "ao" if fc % 2 == 0 else "lg"
                        ps = psum.tile([P, W5], F32, tag=tg,
                                       bufs=(4 if tg == "ao" else 3), name="psk")
                        for dp in range(NDC // 2):
                            nc.tensor.matmul(ps, wk[:, 2 * dp:2 * dp + 2, fc * P:(fc + 1) * P],
                                             yq[:, 2 * dp:2 * dp + 2, :],
                                             start=(dp == 0), stop=(dp == NDC // 2 - 1),
                                             perf_mode=DR)
                        nc.vector.tensor_scalar(KT[:, fc, sl], ps, RSC, bks[:, fc:fc + 1],
                                                op0=ALU.mult, op1=ALU.add)

                    # V: seq-major, DVE descale+bias -> fp8
                    for ks in range(NST):
                        kt = st * NST + ks
                        for dh in range(NDH):
                            tg = "ao" if (ks + dh) % 2 == 0 else "lg"
                            ps = psum.tile([P, W5], F32, tag=tg,
                                           bufs=(4 if tg == "ao" else 3), name="psv")
                            for dp in range(NDC // 2):
                                nc.tensor.matmul(ps, yq[:, 2 * dp:2 * dp + 2, ks * P:(ks + 1) * P],
                                                 wv[:, 2 * dp:2 * dp + 2, dh * W5:(dh + 1) * W5],
                                                 start=(dp == 0), stop=(dp == NDC // 2 - 1),
                                                 perf_mode=DR)
                            nc.vector.scalar_tensor_tensor(
                                V[:, kt, dh * W5:(dh + 1) * W5], ps, RSC,
                                bvb[:, dh * W5:(dh + 1) * W5],
                                op0=ALU.mult, op1=ALU.add)

            # ================= stage B: attention =================
            # Per 512-wide q strip: logitsT[k,q] in psum -> exp to fp8 SBUF
            # (16 [128,512] k-chunk tiles); Z accumulated as a [1,512] psum row
            # (ones stationary); then attn@V per 128-wide q chunk with the exp
            # tiles replayed as DoubleRow stationary operands.
            if KBENCH == "proj":
                continue
            with tc.tile_pool(name=f"stB_{b}", bufs=4) as bp:
                for st in range(NST):
                    sl = slice(st * W5, (st + 1) * W5)
                    exs = expp.tile([P, NKT, W5], F8, tag="exT", bufs=2)
                    xrs = bp.tile([P, NST, D], F32, tag="xrs", bufs=2, name="xrs")
                    nc.sync.dma_start(
                        out=xrs, in_=xr[b, sl, :].rearrange("(c p) d -> p c d", p=P))
                    zps = psum.tile([P, W5], F32, tag="z", bufs=1, name="zps")

                    def z_acc(pr):
                        if KBENCH in ("noz", "noattn"):
                            return
                        nc.tensor.matmul(zps[0:1, :], ones8[:, :, 0:1],
                                         exs[:, 2 * pr:2 * pr + 2, :],
                                         start=(pr == 0), stop=(pr == NPAIR - 1),
                                         perf_mode=DR)

                    for kc in range(NKT):
                        lg = psum.tile([P, W5], F32, tag="lg", bufs=3, name="lg")
                        for fp in range(NFC // 2):
                            nc.tensor.matmul(lg, KT[:, 2 * fp:2 * fp + 2, kc * P:(kc + 1) * P],
                                             QT[:, 2 * fp:2 * fp + 2, sl],
                                             start=(fp == 0), stop=(fp == NFC // 2 - 1),
                                             perf_mode=DR)
                        nc.scalar.activation(exs[:, kc, :], lg, AF.Exp,
                                             bias=ebias[:, 0:1], scale=SM_SCALE)
                        # z pair (kc-1)/2 one pair late so exp is long done
                        if kc % 2 == 1 and kc > 1:
                            z_acc((kc - 1) // 2 - 1)

                    # attn@V, one 128-wide q chunk at a time; first chunk's
                    # matmuls hide the final z pair + Z->rz transpose latency
                    if KBENCH == "noattn":
                        continue
                    rzc = bp.tile([P, NST], F32, tag="rzc", name="rzc")
                    if KBENCH == "noz":
                        nc.vector.memset(rzc, 1.0)
                    for qq in range(NST):
                        qsl = slice(qq * P, (qq + 1) * P)
                        aos = [psum.tile([P, W5], F32, tag="ao", name=f"ao{dh}")
                               for dh in range(NDH)]
                        for pr in range(NPAIR):
                            for dh in range(NDH):
                                nc.tensor.matmul(aos[dh], exs[:, 2 * pr:2 * pr + 2, qsl],
                                                 V[:, 2 * pr:2 * pr + 2, dh * W5:(dh + 1) * W5],
                                                 start=(pr == 0), stop=(pr == NPAIR - 1),
                                                 perf_mode=DR)
                            if qq == 0 and pr == 0 and KBENCH != "noz":
                                z_acc(NPAIR - 1)
                                # Z row -> [128,4] per-partition scalars via a
                                # DRAM bounce (SBUF APs can't express the
                                # transposing view) -> 1/(Z+eps)
                                z2 = bp.tile([P, W5], F32, tag="z2", bufs=1, name="z2")
                                nc.vector.tensor_scalar_add(z2[0:1, :], zps[0:1, :], EPS)
                                zdr = dram.tile([1, W5], F32, tag="zdr")
                                nc.sync.dma_start(out=zdr, in_=z2[0:1, :])
                                zc = bp.tile([P, NST], F32, tag="zc", name="zc")
                                nc.sync.dma_start(
                                    out=zc,
                                    in_=zdr.rearrange("a (q p) -> (a p) q", p=P))
                                nc.vector.reciprocal(rzc, zc)
                        for dh in range(NDH):
                            ob = bp.tile([P, W5], F32, tag="osb", name="ob")
                            nc.vector.scalar_tensor_tensor(
                                ob, aos[dh], rzc[:, qq:qq + 1],
                                xrs[:, qq, dh * W5:(dh + 1) * W5],
                                op0=ALU.mult, op1=ALU.add)
                            nc.sync.dma_start(
                                out=out[b, st * W5 + qq * P:st * W5 + (qq + 1) * P,
                                        dh * W5:(dh + 1) * W5],
                                in_=ob)

    nc.compile()
    return nc


_NC_CACHE = {}


def _get_nc():
    if "nc" not in _NC_CACHE:
        _NC_CACHE["nc"] = _build()
    return _NC_CACHE["nc"]


def _make_in_maps(x, y, Wq, bq, Wk, bk, Wv, bv):
    f8 = mybir.dt.np(F8)
    x = np.asarray(x, dtype=np.float32)
    y = np.asarray(y, dtype=np.float32)
    x8T = np.ascontiguousarray(x.transpose(0, 2, 1)).astype(f8).view(np.uint8)
    y8T = np.ascontiguousarray(y.transpose(0, 2, 1)).astype(f8).view(np.uint8)
    w8q = (np.asarray(Wq, dtype=np.float32) * WSCALE).astype(f8).view(np.uint8)
    w8k = (np.asarray(Wk, dtype=np.float32) * WSCALE).astype(f8).view(np.uint8)
    w8v = (np.asarray(Wv, dtype=np.float32) * WSCALE).astype(f8).view(np.uint8)
    bq = np.ascontiguousarray(np.asarray(bq, dtype=np.float32))
    bk = np.ascontiguousarray(np.asarray(bk, dtype=np.float32))
    bv = np.ascontiguousarray(np.asarray(bv, dtype=np.float32))
    in_maps = []
    for c in range(NCORES):
        sl = slice(c * BPC, (c + 1) * BPC)
        in_maps.append({
            "x8T": np.ascontiguousarray(x8T[sl]),
            "y8T": np.ascontiguousarray(y8T[sl]),
            "xr": np.ascontiguousarray(x[sl]),
            "w8q": w8q, "w8k": w8k, "w8v": w8v,
            "bq": bq, "bk": bk, "bv": bv,
        })
    return in_maps


def kernel(x, y, Wq, bq, Wk, bk, Wv, bv):
    nc = _get_nc()
    in_maps = _make_in_maps(x, y, Wq, bq, Wk, bk, Wv, bv)
    res = run_bass_kernel_spmd(nc, in_maps, core_ids=list(range(NCORES)))
    return np.concatenate([r["out"] for r in res.results], axis=0)


# revision 26
# speedup vs baseline: 4.4095x; 2.7119x over previous
"""Cross-attention (B=16, S=2048, D=1024, fp32) on 8 TRN2 NeuronCores.

Sharding: data-parallel over batch (2 batches per core), weights replicated.

Two operand modes share one structure (KBENCH variant):
  fp8e4 + DoubleRow (default "full"): [128,2,N] k-pair operands, 256-deep
    contraction per PE instruction at 0.5 cycles/row; weights premultiplied
    by 32 on host so W ~ N(0,1) fills the fp8 range (descaled at the
    psum->SBUF cast).  Host-validated rel err 4.5e-3 (gate 2e-2).
  bf16 ("bf16"): plain matmuls, FWL weight loads, no premultiply needed but
    kept for code unity.

Pipeline per batch (all operands SBUF-resident, no DRAM spill):
  stage A: QT[f,s], KT[f,s] (f-major), V[s,f] (seq-major), cast+bias via
           ACT (Q) / DVE (K, V)
  stage B, per 512-wide q strip:
    logitsT[k,q] psum <- KT^T QT ; exp to SBUF (ACT, scale 1/sqrt(D),
      bias -2 keeps exp under fp8e4's 240 max; softmax shift-invariant)
    Z[1,512] psum row <- ones^T exp ; transposed to [128,4] per-partition
      scalars via a DRAM bounce; 1/(Z+eps) on DVE
    attn@V per 128-wide q chunk (exp replayed as stationary), evicted by
      DVE as out = out_u * rz + x (fp32 residual) -> DRAM

fp8/bf16 inputs ship as uint8 DRAM tensors (device bitcast) to keep
jax/PJRT out of exotic dtypes.
"""

import os
import numpy as np
from contextlib import ExitStack

import concourse.bacc as bacc
import concourse.tile as tile
import concourse.mybir as mybir
from concourse.bass_utils import run_bass_kernel_spmd

# variant switch, also used for perf bisection:
#   full | proj | noattn | noz | bf16 | bf16_proj ...
KBENCH_DEFAULT = os.environ.get("KBENCH", "full")

# problem dims (hardcoded per harness contract)
B, S, D = 16, 2048, 1024
NCORES, P = 8, 128
BPC = B // NCORES          # 2 batches per core
NFC = D // P               # 8 feature chunks of 128
NDC = D // P               # 8 contraction chunks of 128
NKT = S // P               # 16 key chunks of 128
W5 = 512
NST = S // W5              # 4 strips of 512
NDH = D // W5              # 2 output-feature halves of 512
WSCALE = 32.0              # host premultiplies weights by this
SM_SCALE = float(1.0 / np.sqrt(D))
EXP_BIAS = -2.0
EPS = 1e-6

F32 = mybir.dt.float32
F8 = mybir.dt.float8e4
DR = mybir.MatmulPerfMode.DoubleRow

AF = mybir.ActivationFunctionType
ALU = mybir.AluOpType


def _is_bf16(kbench):
    return kbench.startswith("bf16")


def _build(KBENCH=None):
    KBENCH = KBENCH_DEFAULT if KBENCH is None else KBENCH
    BF16 = _is_bf16(KBENCH)
    sub = KBENCH.split("_", 1)[1] if (BF16 and "_" in KBENCH) else (
        KBENCH if not BF16 else "full")
    MDT = mybir.dt.bfloat16 if BF16 else F8
    DS = 1 if BF16 else 2            # contraction chunks per matmul
    PM = None if BF16 else DR        # perf_mode
    nb = 2 if BF16 else 1            # operand bytes
    NPAIR = NKT // DS                # kv-chunk groups in attn phase

    nc = bacc.Bacc("TRN2", target_bir_lowering=False, debug=False)

    # All operand tensors are pre-tiled on HOST into the exact SBUF layout so
    # every DMA moves kilobyte-contiguous runs per partition (a naive
    # "(dc p) s" gather costs ~1024 descriptors of 512B per strip and made
    # stage A DMA-descriptor-bound).
    u8 = mybir.dt.uint8
    x8T = nc.dram_tensor("x8T", [BPC, NST, P, NDC * W5 * nb], u8,
                         kind="ExternalInput").ap().bitcast(MDT)
    y8T = nc.dram_tensor("y8T", [BPC, NST, P, NDC * W5 * nb], u8,
                         kind="ExternalInput").ap().bitcast(MDT)
    xr = nc.dram_tensor("xr", [BPC, S, D], F32, kind="ExternalInput").ap()
    w8q = nc.dram_tensor("w8q", [P, NDC * D * nb], u8, kind="ExternalInput").ap().bitcast(MDT)
    w8k = nc.dram_tensor("w8k", [P, NDC * D * nb], u8, kind="ExternalInput").ap().bitcast(MDT)
    w8v = nc.dram_tensor("w8v", [P, NDC * D * nb], u8, kind="ExternalInput").ap().bitcast(MDT)
    bq = nc.dram_tensor("bq", [P, NFC], F32, kind="ExternalInput").ap()
    bk = nc.dram_tensor("bk", [P, NFC], F32, kind="ExternalInput").ap()
    bv = nc.dram_tensor("bv", [D], F32, kind="ExternalInput").ap()
    out = nc.dram_tensor("out", [BPC, S, D], F32, kind="ExternalOutput").ap()

    RSC = float(1.0 / WSCALE)

    with tile.TileContext(nc) as tc, ExitStack() as ctx:
        const = ctx.enter_context(tc.tile_pool(name="const", bufs=1))
        kvp = ctx.enter_context(tc.tile_pool(name="kvp", bufs=1))
        expp = ctx.enter_context(tc.tile_pool(name="expp", bufs=1))
        psum = ctx.enter_context(tc.tile_pool(name="psum", bufs=4, space="PSUM"))
        dram = ctx.enter_context(tc.tile_pool(name="dram", bufs=2, space="DRAM"))

        # ---- constants
        onesf = const.tile([P, 2, 16], F32)
        nc.vector.memset(onesf, 1.0)
        ones8 = const.tile([P, 2, 16], MDT)
        nc.vector.tensor_copy(ones8, onesf)
        bqs = const.tile([P, NFC], F32)
        nc.gpsimd.dma_start(out=bqs, in_=bq)
        bks = const.tile([P, NFC], F32)
        nc.gpsimd.dma_start(out=bks, in_=bk)
        import concourse.bass as bass
        bvb = const.tile([P, D], F32)
        bv1 = bv.rearrange("(a d) -> a d", a=1)
        bv_bcast = bass.AP(tensor=bv1.tensor, offset=bv1.offset,
                           ap=[[0, P]] + list(bv1.ap[1:]))
        nc.gpsimd.dma_start(out=bvb, in_=bv_bcast)
        ebias = const.tile([P, 1], F32)
        nc.vector.memset(ebias, EXP_BIAS)

        # ---- projection weights, resident [P, dc, f] (host pre-tiled)
        wq = const.tile([P, NDC, D], MDT, name="wq")
        nc.sync.dma_start(out=wq, in_=w8q.rearrange("p (dc f) -> p dc f", dc=NDC))
        wk = const.tile([P, NDC, D], MDT, name="wk")
        nc.sync.dma_start(out=wk, in_=w8k.rearrange("p (dc f) -> p dc f", dc=NDC))
        wv = const.tile([P, NDC, D], MDT, name="wv")
        nc.sync.dma_start(out=wv, in_=w8v.rearrange("p (dc f) -> p dc f", dc=NDC))

        for b in range(BPC):
            # resident operands for this batch
            QT = kvp.tile([P, NFC, S], MDT, tag="QT")     # f-major
            KT = kvp.tile([P, NFC, S], MDT, tag="KT")     # f-major
            V = kvp.tile([P, NKT, D], MDT, tag="V")       # seq-major

            # ================= stage A: projections =================
            with tc.tile_pool(name=f"stA_{b}", bufs=2) as strips:
                for st in range(NST):
                    sl = slice(st * W5, (st + 1) * W5)
                    xq = strips.tile([P, NDC, W5], MDT, tag="xq", name="xq",
                                     bufs=(1 if BF16 else 2))
                    nc.sync.dma_start(out=xq, in_=x8T[b, st].rearrange("p (dc s) -> p dc s", dc=NDC))
                    yq = strips.tile([P, NDC, W5], MDT, tag="yq", name="yq",
                                     bufs=(1 if BF16 else 2))
                    nc.sync.dma_start(out=yq, in_=y8T[b, st].rearrange("p (dc s) -> p dc s", dc=NDC))

                    # Q: f-major, ACT descale+bias
                    for fc in range(NFC):
                        tg = "ao" if fc % 2 == 0 else "lg"
                        ps = psum.tile([P, W5], F32, tag=tg,
                                       bufs=(4 if tg == "ao" else 3), name="psq")
                        for dp in range(NDC // DS):
                            nc.tensor.matmul(ps, wq[:, DS * dp:DS * dp + DS, fc * P:(fc + 1) * P],
                                             xq[:, DS * dp:DS * dp + DS, :],
                                             start=(dp == 0), stop=(dp == NDC // DS - 1),
                                             perf_mode=PM)
                        nc.scalar.activation(QT[:, fc, sl], ps, AF.Identity,
                                             bias=bqs[:, fc:fc + 1], scale=RSC)

                    # K: f-major, DVE descale+bias
                    for fc in range(NFC):
                        tg = "ao" if fc % 2 == 0 else "lg"
                        ps = psum.tile([P, W5], F32, tag=tg,
                                       bufs=(4 if tg == "ao" else 3), name="psk")
                        for dp in range(NDC // DS):
                            nc.tensor.matmul(ps, wk[:, DS * dp:DS * dp + DS, fc * P:(fc + 1) * P],
                                             yq[:, DS * dp:DS * dp + DS, :],
                                             start=(dp == 0), stop=(dp == NDC // DS - 1),
                                             perf_mode=PM)
                        nc.vector.tensor_scalar(KT[:, fc, sl], ps, RSC, bks[:, fc:fc + 1],
                                                op0=ALU.mult, op1=ALU.add)

                    # V: seq-major, DVE descale+bias
                    for ks in range(NST):
                        kt = st * NST + ks
                        for dh in range(NDH):
                            tg = "ao" if (ks + dh) % 2 == 0 else "lg"
                            ps = psum.tile([P, W5], F32, tag=tg,
                                           bufs=(4 if tg == "ao" else 3), name="psv")
                            for dp in range(NDC // DS):
                                nc.tensor.matmul(ps, yq[:, DS * dp:DS * dp + DS, ks * P:(ks + 1) * P],
                                                 wv[:, DS * dp:DS * dp + DS, dh * W5:(dh + 1) * W5],
                                                 start=(dp == 0), stop=(dp == NDC // DS - 1),
                                                 perf_mode=PM)
                            nc.vector.scalar_tensor_tensor(
                                V[:, kt, dh * W5:(dh + 1) * W5], ps, RSC,
                                bvb[:, dh * W5:(dh + 1) * W5],
                                op0=ALU.mult, op1=ALU.add)

            # ================= stage B: attention =================
            if sub == "proj":
                continue
            with tc.tile_pool(name=f"stB_{b}", bufs=4) as bp:
                for st in range(NST):
                    sl = slice(st * W5, (st + 1) * W5)
                    exs = expp.tile([P, NKT, W5], MDT, tag="exT",
                                    bufs=(1 if BF16 else 2))
                    xrs = bp.tile([P, NST, D], F32, tag="xrs",
                                  bufs=(1 if BF16 else 2), name="xrs")
                    nc.sync.dma_start(
                        out=xrs, in_=xr[b, sl, :].rearrange("(c p) d -> p c d", p=P))
                    zps = psum.tile([P, W5], F32, tag="z", bufs=1, name="zps")

                    def z_acc(pr):
                        if sub in ("noz", "noattn"):
                            return
                        nc.tensor.matmul(zps[0:1, :], ones8[:, 0:DS, 0:1],
                                         exs[:, DS * pr:DS * pr + DS, :],
                                         start=(pr == 0), stop=(pr == NPAIR - 1),
                                         perf_mode=PM)

                    for kc in range(NKT):
                        lg = psum.tile([P, W5], F32, tag="lg", bufs=3, name="lg")
                        for fp in range(NFC // DS):
                            nc.tensor.matmul(lg, KT[:, DS * fp:DS * fp + DS, kc * P:(kc + 1) * P],
                                             QT[:, DS * fp:DS * fp + DS, sl],
                                             start=(fp == 0), stop=(fp == NFC // DS - 1),
                                             perf_mode=PM)
                        nc.scalar.activation(exs[:, kc, :], lg, AF.Exp,
                                             bias=ebias[:, 0:1], scale=SM_SCALE)
                        # z group (kc//DS - 1) one group late so exp is done
                        if kc % DS == DS - 1 and kc >= DS:
                            z_acc(kc // DS - 1)

                    # attn@V, one 128-wide q chunk at a time; first chunk's
                    # matmuls hide the final z group + Z->rz transpose latency
                    if sub == "noattn":
                        continue
                    rzc = bp.tile([P, NST], F32, tag="rzc", name="rzc")
                    if sub == "noz":
                        nc.vector.memset(rzc, 1.0)
                    for qq in range(NST):
                        qsl = slice(qq * P, (qq + 1) * P)
                        aos = [psum.tile([P, W5], F32, tag="ao", name=f"ao{dh}")
                               for dh in range(NDH)]
                        for pr in range(NPAIR):
                            for dh in range(NDH):
                                nc.tensor.matmul(aos[dh], exs[:, DS * pr:DS * pr + DS, qsl],
                                                 V[:, DS * pr:DS * pr + DS, dh * W5:(dh + 1) * W5],
                                                 start=(pr == 0), stop=(pr == NPAIR - 1),
                                                 perf_mode=PM)
                            if qq == 0 and pr == 0 and sub != "noz":
                                z_acc(NPAIR - 1)
                                # Z row -> [128,4] per-partition scalars via a
                                # DRAM bounce (SBUF APs can't express the
                                # transposing view) -> 1/(Z+eps)
                                z2 = bp.tile([P, W5], F32, tag="z2", bufs=1, name="z2")
                                nc.vector.tensor_scalar_add(z2[0:1, :], zps[0:1, :], EPS)
                                zdr = dram.tile([1, W5], F32, tag="zdr")
                                nc.sync.dma_start(out=zdr, in_=z2[0:1, :])
                                zc = bp.tile([P, NST], F32, tag="zc", name="zc")
                                nc.sync.dma_start(
                                    out=zc,
                                    in_=zdr.rearrange("a (q p) -> (a p) q", p=P))
                                nc.vector.reciprocal(rzc, zc)
                        for dh in range(NDH):
                            ob = bp.tile([P, W5], F32, tag="osb", name="ob")
                            nc.vector.scalar_tensor_tensor(
                                ob, aos[dh], rzc[:, qq:qq + 1],
                                xrs[:, qq, dh * W5:(dh + 1) * W5],
                                op0=ALU.mult, op1=ALU.add)
                            nc.sync.dma_start(
                                out=out[b, st * W5 + qq * P:st * W5 + (qq + 1) * P,
                                        dh * W5:(dh + 1) * W5],
                                in_=ob)

    nc.compile()
    return nc


def _build_pe(KBENCH="pairev"):
    """Paired-eviction design: every ACT/DVE psum eviction covers TWO psum
    banks ([128,1024]) and output DMAs write full-D rows.  PSUM: pp pairs
    (2x2 banks) + lg (3) + z (1) = 8 banks.

    Q/K biases are folded per-partition via stride-0 broadcast APs; V bias
    via the bvb row tile. Exp stays per-bank (lg tiles are per-k-chunk).
    """
    import concourse.bass as bass
    BF16 = _is_bf16(KBENCH)
    MDT = mybir.dt.bfloat16 if BF16 else F8
    DS = 1 if BF16 else 2
    PM = None if BF16 else DR
    nb = 2 if BF16 else 1
    NPAIR = NKT // DS

    nc = bacc.Bacc("TRN2", target_bir_lowering=False, debug=False)

    u8 = mybir.dt.uint8
    x8T = nc.dram_tensor("x8T", [BPC, NST, P, NDC * W5 * nb], u8,
                         kind="ExternalInput").ap().bitcast(MDT)
    y8T = nc.dram_tensor("y8T", [BPC, NST, P, NDC * W5 * nb], u8,
                         kind="ExternalInput").ap().bitcast(MDT)
    xr = nc.dram_tensor("xr", [BPC, S, D], F32, kind="ExternalInput").ap()
    w8q = nc.dram_tensor("w8q", [P, NDC * D * nb], u8, kind="ExternalInput").ap().bitcast(MDT)
    w8k = nc.dram_tensor("w8k", [P, NDC * D * nb], u8, kind="ExternalInput").ap().bitcast(MDT)
    w8v = nc.dram_tensor("w8v", [P, NDC * D * nb], u8, kind="ExternalInput").ap().bitcast(MDT)
    bq = nc.dram_tensor("bq", [P, NFC], F32, kind="ExternalInput").ap()
    bk = nc.dram_tensor("bk", [P, NFC], F32, kind="ExternalInput").ap()
    bv = nc.dram_tensor("bv", [D], F32, kind="ExternalInput").ap()
    out = nc.dram_tensor("out", [BPC, S, D], F32, kind="ExternalOutput").ap()

    RSC = float(1.0 / WSCALE)

    def bc2(t, fc):
        # [P, 2, W5] stride-0 view of bias columns fc, fc+1 of a [P, NFC] tile
        a = t[:, fc:fc + 2]
        return bass.AP(tensor=a.tensor, offset=a.offset,
                       ap=list(a.ap) + [[0, W5]])

    with tile.TileContext(nc) as tc, ExitStack() as ctx:
        const = ctx.enter_context(tc.tile_pool(name="const", bufs=1))
        kvp = ctx.enter_context(tc.tile_pool(name="kvp", bufs=1))
        expp = ctx.enter_context(tc.tile_pool(name="expp", bufs=1))
        psum = ctx.enter_context(tc.tile_pool(name="psum", bufs=2, space="PSUM"))
        dram = ctx.enter_context(tc.tile_pool(name="dram", bufs=2, space="DRAM"))

        onesf = const.tile([P, 2, 16], F32)
        nc.vector.memset(onesf, 1.0)
        ones8 = const.tile([P, 2, 16], MDT)
        nc.vector.tensor_copy(ones8, onesf)
        bqs = const.tile([P, NFC], F32)
        nc.gpsimd.dma_start(out=bqs, in_=bq)
        bks = const.tile([P, NFC], F32)
        nc.gpsimd.dma_start(out=bks, in_=bk)
        bvb = const.tile([P, D], F32)
        bv1 = bv.rearrange("(a d) -> a d", a=1)
        bv_bcast = bass.AP(tensor=bv1.tensor, offset=bv1.offset,
                           ap=[[0, P]] + list(bv1.ap[1:]))
        nc.gpsimd.dma_start(out=bvb, in_=bv_bcast)
        ebias = const.tile([P, 1], F32)
        nc.vector.memset(ebias, EXP_BIAS)

        wq = const.tile([P, NDC, D], MDT, name="wq")
        nc.sync.dma_start(out=wq, in_=w8q.rearrange("p (dc f) -> p dc f", dc=NDC))
        wk = const.tile([P, NDC, D], MDT, name="wk")
        nc.sync.dma_start(out=wk, in_=w8k.rearrange("p (dc f) -> p dc f", dc=NDC))
        wv = const.tile([P, NDC, D], MDT, name="wv")
        nc.sync.dma_start(out=wv, in_=w8v.rearrange("p (dc f) -> p dc f", dc=NDC))

        for b in range(BPC):
            QT = kvp.tile([P, NFC, S], MDT, tag="QT")
            KT = kvp.tile([P, NFC, S], MDT, tag="KT")
            V = kvp.tile([P, NKT, D], MDT, tag="V")

            # ---------------- stage A ----------------
            with tc.tile_pool(name=f"stA_{b}", bufs=2) as strips:
                for st in range(NST):
                    sl = slice(st * W5, (st + 1) * W5)
                    xq = strips.tile([P, NDC, W5], MDT, tag="xq", name="xq", bufs=2)
                    nc.sync.dma_start(out=xq, in_=x8T[b, st].rearrange("p (dc s) -> p dc s", dc=NDC))
                    yq = strips.tile([P, NDC, W5], MDT, tag="yq", name="yq", bufs=2)
                    nc.sync.dma_start(out=yq, in_=y8T[b, st].rearrange("p (dc s) -> p dc s", dc=NDC))

                    # Q then K: evict two fc banks per instr
                    for w_, dst, bt in ((wq, QT, bqs), (wk, KT, bks)):
                        for fc2 in range(NFC // 2):
                            pp = psum.tile([P, 2, W5], F32, tag="pp", bufs=2, name="pp")
                            for j in range(2):
                                fc = 2 * fc2 + j
                                for dp in range(NDC // DS):
                                    nc.tensor.matmul(
                                        pp[:, j, :],
                                        w_[:, DS * dp:DS * dp + DS, fc * P:(fc + 1) * P],
                                        xq[:, DS * dp:DS * dp + DS, :] if w_ is wq
                                        else yq[:, DS * dp:DS * dp + DS, :],
                                        start=(dp == 0), stop=(dp == NDC // DS - 1),
                                        perf_mode=PM)
                            # stt with stride-0 bias broadcast (honest bias);
                            # Q on ACT-side? both evictors are DVE-class ops,
                            # so split Q->ACT is not available for stt; keep
                            # Q and K on DVE, V below also DVE, exp on ACT.
                            nc.vector.scalar_tensor_tensor(
                                dst[:, 2 * fc2:2 * fc2 + 2, sl], pp, RSC,
                                bc2(bt, 2 * fc2), op0=ALU.mult, op1=ALU.add)

                    # V: evict both dh halves of a seq chunk per instr
                    for ks in range(NST):
                        kt = st * NST + ks
                        pp = psum.tile([P, 2, W5], F32, tag="pp", bufs=2, name="ppv")
                        for dh in range(NDH):
                            for dp in range(NDC // DS):
                                nc.tensor.matmul(pp[:, dh, :],
                                                 yq[:, DS * dp:DS * dp + DS, ks * P:(ks + 1) * P],
                                                 wv[:, DS * dp:DS * dp + DS, dh * W5:(dh + 1) * W5],
                                                 start=(dp == 0), stop=(dp == NDC // DS - 1),
                                                 perf_mode=PM)
                        nc.vector.scalar_tensor_tensor(
                            V[:, kt, :].rearrange("p (t w) -> p t w", t=2), pp, RSC,
                            bvb.rearrange("p (t w) -> p t w", t=2),
                            op0=ALU.mult, op1=ALU.add)

            # ---------------- stage B ----------------
            with tc.tile_pool(name=f"stB_{b}", bufs=4) as bp:
                for st in range(NST):
                    sl = slice(st * W5, (st + 1) * W5)
                    exs = expp.tile([P, NKT, W5], MDT, tag="exT", bufs=2)
                    xrs = bp.tile([P, NST, D], F32, tag="xrs", bufs=2, name="xrs")
                    nc.sync.dma_start(
                        out=xrs, in_=xr[b, sl, :].rearrange("(c p) d -> p c d", p=P))
                    zps = psum.tile([P, W5], F32, tag="z", bufs=1, name="zps")

                    def z_acc(pr):
                        nc.tensor.matmul(zps[0:1, :], ones8[:, 0:DS, 0:1],
                                         exs[:, DS * pr:DS * pr + DS, :],
                                         start=(pr == 0), stop=(pr == NPAIR - 1),
                                         perf_mode=PM)

                    for kc in range(NKT):
                        lg = psum.tile([P, W5], F32, tag="lg", bufs=3, name="lg")
                        for fp in range(NFC // DS):
                            nc.tensor.matmul(lg, KT[:, DS * fp:DS * fp + DS, kc * P:(kc + 1) * P],
                                             QT[:, DS * fp:DS * fp + DS, sl],
                                             start=(fp == 0), stop=(fp == NFC // DS - 1),
                                             perf_mode=PM)
                        nc.scalar.activation(exs[:, kc, :], lg, AF.Exp,
                                             bias=ebias[:, 0:1], scale=SM_SCALE)
                        if kc % DS == DS - 1 and kc >= DS:
                            z_acc(kc // DS - 1)

                    rzc = bp.tile([P, NST], F32, tag="rzc", name="rzc")
                    for qq in range(NST):
                        qsl = slice(qq * P, (qq + 1) * P)
                        aos = psum.tile([P, 2, W5], F32, tag="pp", bufs=2, name="aos")
                        for pr in range(NPAIR):
                            for dh in range(NDH):
                                nc.tensor.matmul(aos[:, dh, :], exs[:, DS * pr:DS * pr + DS, qsl],
                                                 V[:, DS * pr:DS * pr + DS, dh * W5:(dh + 1) * W5],
                                                 start=(pr == 0), stop=(pr == NPAIR - 1),
                                                 perf_mode=PM)
                            if qq == 0 and pr == 0:
                                z_acc(NPAIR - 1)
                                z2 = bp.tile([P, W5], F32, tag="z2", bufs=1, name="z2")
                                nc.vector.tensor_scalar_add(z2[0:1, :], zps[0:1, :], EPS)
                                zdr = dram.tile([1, W5], F32, tag="zdr")
                                nc.sync.dma_start(out=zdr, in_=z2[0:1, :])
                                zc = bp.tile([P, NST], F32, tag="zc", name="zc")
                                nc.sync.dma_start(
                                    out=zc,
                                    in_=zdr.rearrange("a (q p) -> (a p) q", p=P))
                                nc.vector.reciprocal(rzc, zc)
                        ob = bp.tile([P, D], F32, tag="osb", name="ob")
                        nc.vector.scalar_tensor_tensor(
                            ob.rearrange("p (t w) -> p t w", t=2), aos, rzc[:, qq:qq + 1],
                            xrs[:, qq, :].rearrange("p (t w) -> p t w", t=2),
                            op0=ALU.mult, op1=ALU.add)
                        nc.sync.dma_start(
                            out=out[b, st * W5 + qq * P:st * W5 + (qq + 1) * P, :],
                            in_=ob)

    nc.compile()
    return nc


_NC_CACHE = {}


def _get_nc(KBENCH=None):
    key = KBENCH_DEFAULT if KBENCH is None else KBENCH
    if key not in _NC_CACHE:
        if key in ("full", "pairev") or key.endswith("pairev"):
            _NC_CACHE[key] = _build_pe(key)
        else:
            _NC_CACHE[key] = _build(key)
    return _NC_CACHE[key]


def _tile_strips(a, mdt):
    """[B, S, D] fp32 -> [B, NST, P, NDC*W5*nb] u8: strip-tile layout where
    tile[p, dc, s'] = a[b, st*W5+s', dc*P+p] (feature-major per strip)."""
    B_ = a.shape[0]
    t = a.astype(mdt)                                    # [B, S, D]
    t = t.reshape(B_, NST, W5, NDC, P)                   # S=(st,s'), D=(dc,p)
    t = np.ascontiguousarray(t.transpose(0, 1, 4, 3, 2))  # [B, st, p, dc, s']
    return t.reshape(B_, NST, P, -1).view(np.uint8).reshape(B_, NST, P, -1)


def _tile_w(w, mdt):
    """[D, D] fp32 -> [P, NDC*D*nb] u8 with tile[p, dc, f] = w[dc*P+p, f]."""
    t = (np.asarray(w, dtype=np.float32) * WSCALE).astype(mdt)
    t = np.ascontiguousarray(t.reshape(NDC, P, D).transpose(1, 0, 2))
    return t.reshape(P, -1).view(np.uint8).reshape(P, -1)


def _make_in_maps(x, y, Wq, bq, Wk, bk, Wv, bv, KBENCH=None):
    kb = KBENCH_DEFAULT if KBENCH is None else KBENCH
    mdt = np.dtype(mybir.dt.np(mybir.dt.bfloat16 if _is_bf16(kb) else F8))
    x = np.asarray(x, dtype=np.float32)
    y = np.asarray(y, dtype=np.float32)
    x8T = _tile_strips(x, mdt)
    y8T = _tile_strips(y, mdt)
    w8q = _tile_w(Wq, mdt)
    w8k = _tile_w(Wk, mdt)
    w8v = _tile_w(Wv, mdt)
    bq = np.ascontiguousarray(np.asarray(bq, dtype=np.float32).reshape(NFC, P).T)
    bk = np.ascontiguousarray(np.asarray(bk, dtype=np.float32).reshape(NFC, P).T)
    bv = np.ascontiguousarray(np.asarray(bv, dtype=np.float32))
    in_maps = []
    for c in range(NCORES):
        sl = slice(c * BPC, (c + 1) * BPC)
        in_maps.append({
            "x8T": np.ascontiguousarray(x8T[sl]),
            "y8T": np.ascontiguousarray(y8T[sl]),
            "xr": np.ascontiguousarray(x[sl]),
            "w8q": w8q, "w8k": w8k, "w8v": w8v,
            "bq": bq, "bk": bk, "bv": bv,
        })
    return in_maps


def kernel(x, y, Wq, bq, Wk, bk, Wv, bv):
    nc = _get_nc()
    in_maps = _make_in_maps(x, y, Wq, bq, Wk, bk, Wv, bv)
    res = run_bass_kernel_spmd(nc, in_maps, core_ids=list(range(NCORES)))
    return np.concatenate([r["out"] for r in res.results], axis=0)


# revision 34
# speedup vs baseline: 5.6527x; 1.2820x over previous
"""Cross-attention (B=16, S=2048, D=1024, fp32) on 8 TRN2 NeuronCores.

Sharding: data-parallel over batch (2 batches per core), weights replicated.

All five matmuls run as fp8e4 DoubleRow ([128,2,N] k-pair operands: 256-deep
contraction per PE instruction at 0.5 cycles/row, half the instruction and
LDWEIGHTS count of an fp32r kernel).  Weights are premultiplied by 32 on
host so W ~ N(0,1) fills the fp8 range (descaled at the psum->SBUF cast);
exp gets a -2 bias so values stay under TRN fp8e4's +-240 max (softmax is
shift-invariant).  Measured rel err 3.6e-3 vs the 2e-2 gate.

Shipped design = _build_pe ("paired eviction"), measured 856 us vs the
1466 us fp32r baseline.  Key lessons baked in:
  - ACT/DVE instructions carry ~us-scale fixed overhead on this part, so
    every psum eviction covers TWO banks ([128,1024] ops) and output DMA
    writes full-D rows.  PSUM: 2x2-bank "pp" pairs + 3 logit banks + 1 Z
    bank = 8.
  - All operand tensors are pre-tiled on HOST into exact SBUF tile layout
    (kilobyte-contiguous per partition) so no DMA is descriptor-bound.
  - Everything is SBUF-resident in fp8 (Q/K/V 2MB per batch, exp 1MB per
    strip); y is loaded once and feeds both K and V.
  - Z is a [1,512] psum row (ones-stationary matmul over exp), transposed
    to [128,4] per-partition scalars via a DRAM bounce, then 1/(Z+eps) on
    DVE; its latency hides under the first attention chunk's matmuls.
  - Q/K biases fold into the paired evictions via stride-0 broadcast APs.

fp8 inputs ship as uint8 DRAM tensors (device bitcast) to keep jax/PJRT
out of exotic dtypes.  KBENCH env/args select perf-bisect variants
(proj/noattn/noz/bf16/sep-eviction); default "full" is the shipped kernel.
"""

import os
import numpy as np
from contextlib import ExitStack

import concourse.bacc as bacc
import concourse.tile as tile
import concourse.mybir as mybir
from concourse.bass_utils import run_bass_kernel_spmd

# variant switch, also used for perf bisection:
#   full | proj | noattn | noz | bf16 | bf16_proj ...
KBENCH_DEFAULT = os.environ.get("KBENCH", "full")

# problem dims (hardcoded per harness contract)
B, S, D = 16, 2048, 1024
NCORES, P = 8, 128
BPC = B // NCORES          # 2 batches per core
NFC = D // P               # 8 feature chunks of 128
NDC = D // P               # 8 contraction chunks of 128
NKT = S // P               # 16 key chunks of 128
W5 = 512
NST = S // W5              # 4 strips of 512
NDH = D // W5              # 2 output-feature halves of 512
WSCALE = 32.0              # host premultiplies weights by this
SM_SCALE = float(1.0 / np.sqrt(D))
EXP_BIAS = -2.0
EPS = 1e-6

F32 = mybir.dt.float32
F8 = mybir.dt.float8e4
DR = mybir.MatmulPerfMode.DoubleRow

AF = mybir.ActivationFunctionType
ALU = mybir.AluOpType


def _is_bf16(kbench):
    return kbench.startswith("bf16")


def _build(KBENCH=None):
    KBENCH = KBENCH_DEFAULT if KBENCH is None else KBENCH
    BF16 = _is_bf16(KBENCH)
    sub = KBENCH.split("_", 1)[1] if (BF16 and "_" in KBENCH) else (
        KBENCH if not BF16 else "full")
    MDT = mybir.dt.bfloat16 if BF16 else F8
    DS = 1 if BF16 else 2            # contraction chunks per matmul
    PM = None if BF16 else DR        # perf_mode
    nb = 2 if BF16 else 1            # operand bytes
    NPAIR = NKT // DS                # kv-chunk groups in attn phase

    nc = bacc.Bacc("TRN2", target_bir_lowering=False, debug=False)

    # All operand tensors are pre-tiled on HOST into the exact SBUF layout so
    # every DMA moves kilobyte-contiguous runs per partition (a naive
    # "(dc p) s" gather costs ~1024 descriptors of 512B per strip and made
    # stage A DMA-descriptor-bound).
    u8 = mybir.dt.uint8
    x8T = nc.dram_tensor("x8T", [BPC, NST, P, NDC * W5 * nb], u8,
                         kind="ExternalInput").ap().bitcast(MDT)
    y8T = nc.dram_tensor("y8T", [BPC, NST, P, NDC * W5 * nb], u8,
                         kind="ExternalInput").ap().bitcast(MDT)
    xr = nc.dram_tensor("xr", [BPC, S, D], F32, kind="ExternalInput").ap()
    w8q = nc.dram_tensor("w8q", [P, NDC * D * nb], u8, kind="ExternalInput").ap().bitcast(MDT)
    w8k = nc.dram_tensor("w8k", [P, NDC * D * nb], u8, kind="ExternalInput").ap().bitcast(MDT)
    w8v = nc.dram_tensor("w8v", [P, NDC * D * nb], u8, kind="ExternalInput").ap().bitcast(MDT)
    bq = nc.dram_tensor("bq", [P, NFC], F32, kind="ExternalInput").ap()
    bk = nc.dram_tensor("bk", [P, NFC], F32, kind="ExternalInput").ap()
    bv = nc.dram_tensor("bv", [D], F32, kind="ExternalInput").ap()
    out = nc.dram_tensor("out", [BPC, S, D], F32, kind="ExternalOutput").ap()

    RSC = float(1.0 / WSCALE)

    with tile.TileContext(nc) as tc, ExitStack() as ctx:
        const = ctx.enter_context(tc.tile_pool(name="const", bufs=1))
        kvp = ctx.enter_context(tc.tile_pool(name="kvp", bufs=1))
        expp = ctx.enter_context(tc.tile_pool(name="expp", bufs=1))
        psum = ctx.enter_context(tc.tile_pool(name="psum", bufs=4, space="PSUM"))
        dram = ctx.enter_context(tc.tile_pool(name="dram", bufs=2, space="DRAM"))

        # ---- constants
        onesf = const.tile([P, 2, 16], F32)
        nc.vector.memset(onesf, 1.0)
        ones8 = const.tile([P, 2, 16], MDT)
        nc.vector.tensor_copy(ones8, onesf)
        bqs = const.tile([P, NFC], F32)
        nc.gpsimd.dma_start(out=bqs, in_=bq)
        bks = const.tile([P, NFC], F32)
        nc.gpsimd.dma_start(out=bks, in_=bk)
        import concourse.bass as bass
        bvb = const.tile([P, D], F32)
        bv1 = bv.rearrange("(a d) -> a d", a=1)
        bv_bcast = bass.AP(tensor=bv1.tensor, offset=bv1.offset,
                           ap=[[0, P]] + list(bv1.ap[1:]))
        nc.gpsimd.dma_start(out=bvb, in_=bv_bcast)
        ebias = const.tile([P, 1], F32)
        nc.vector.memset(ebias, EXP_BIAS)

        # ---- projection weights, resident [P, dc, f] (host pre-tiled)
        wq = const.tile([P, NDC, D], MDT, name="wq")
        nc.sync.dma_start(out=wq, in_=w8q.rearrange("p (dc f) -> p dc f", dc=NDC))
        wk = const.tile([P, NDC, D], MDT, name="wk")
        nc.sync.dma_start(out=wk, in_=w8k.rearrange("p (dc f) -> p dc f", dc=NDC))
        wv = const.tile([P, NDC, D], MDT, name="wv")
        nc.sync.dma_start(out=wv, in_=w8v.rearrange("p (dc f) -> p dc f", dc=NDC))

        for b in range(BPC):
            # resident operands for this batch
            QT = kvp.tile([P, NFC, S], MDT, tag="QT")     # f-major
            KT = kvp.tile([P, NFC, S], MDT, tag="KT")     # f-major
            V = kvp.tile([P, NKT, D], MDT, tag="V")       # seq-major

            # ================= stage A: projections =================
            with tc.tile_pool(name=f"stA_{b}", bufs=2) as strips:
                for st in range(NST):
                    sl = slice(st * W5, (st + 1) * W5)
                    xq = strips.tile([P, NDC, W5], MDT, tag="xq", name="xq",
                                     bufs=(1 if BF16 else 2))
                    nc.sync.dma_start(out=xq, in_=x8T[b, st].rearrange("p (dc s) -> p dc s", dc=NDC))
                    yq = strips.tile([P, NDC, W5], MDT, tag="yq", name="yq",
                                     bufs=(1 if BF16 else 2))
                    nc.sync.dma_start(out=yq, in_=y8T[b, st].rearrange("p (dc s) -> p dc s", dc=NDC))

                    # Q: f-major, ACT descale+bias
                    for fc in range(NFC):
                        tg = "ao" if fc % 2 == 0 else "lg"
                        ps = psum.tile([P, W5], F32, tag=tg,
                                       bufs=(4 if tg == "ao" else 3), name="psq")
                        for dp in range(NDC // DS):
                            nc.tensor.matmul(ps, wq[:, DS * dp:DS * dp + DS, fc * P:(fc + 1) * P],
                                             xq[:, DS * dp:DS * dp + DS, :],
                                             start=(dp == 0), stop=(dp == NDC // DS - 1),
                                             perf_mode=PM)
                        nc.scalar.activation(QT[:, fc, sl], ps, AF.Identity,
                                             bias=bqs[:, fc:fc + 1], scale=RSC)

                    # K: f-major, DVE descale+bias
                    for fc in range(NFC):
                        tg = "ao" if fc % 2 == 0 else "lg"
                        ps = psum.tile([P, W5], F32, tag=tg,
                                       bufs=(4 if tg == "ao" else 3), name="psk")
                        for dp in range(NDC // DS):
                            nc.tensor.matmul(ps, wk[:, DS * dp:DS * dp + DS, fc * P:(fc + 1) * P],
                                             yq[:, DS * dp:DS * dp + DS, :],
                                             start=(dp == 0), stop=(dp == NDC // DS - 1),
                                             perf_mode=PM)
                        nc.vector.tensor_scalar(KT[:, fc, sl], ps, RSC, bks[:, fc:fc + 1],
                                                op0=ALU.mult, op1=ALU.add)

                    # V: seq-major, DVE descale+bias
                    for ks in range(NST):
                        kt = st * NST + ks
                        for dh in range(NDH):
                            tg = "ao" if (ks + dh) % 2 == 0 else "lg"
                            ps = psum.tile([P, W5], F32, tag=tg,
                                           bufs=(4 if tg == "ao" else 3), name="psv")
                            for dp in range(NDC // DS):
                                nc.tensor.matmul(ps, yq[:, DS * dp:DS * dp + DS, ks * P:(ks + 1) * P],
                                                 wv[:, DS * dp:DS * dp + DS, dh * W5:(dh + 1) * W5],
                                                 start=(dp == 0), stop=(dp == NDC // DS - 1),
                                                 perf_mode=PM)
                            nc.vector.scalar_tensor_tensor(
                                V[:, kt, dh * W5:(dh + 1) * W5], ps, RSC,
                                bvb[:, dh * W5:(dh + 1) * W5],
                                op0=ALU.mult, op1=ALU.add)

            # ================= stage B: attention =================
            if sub == "proj":
                continue
            with tc.tile_pool(name=f"stB_{b}", bufs=4) as bp:
                for st in range(NST):
                    sl = slice(st * W5, (st + 1) * W5)
                    exs = expp.tile([P, NKT, W5], MDT, tag="exT",
                                    bufs=(1 if BF16 else 2))
                    xrs = bp.tile([P, NST, D], F32, tag="xrs",
                                  bufs=(1 if BF16 else 2), name="xrs")
                    nc.sync.dma_start(
                        out=xrs, in_=xr[b, sl, :].rearrange("(c p) d -> p c d", p=P))
                    zps = psum.tile([P, W5], F32, tag="z", bufs=1, name="zps")

                    def z_acc(pr):
                        if sub in ("noz", "noattn"):
                            return
                        nc.tensor.matmul(zps[0:1, :], ones8[:, 0:DS, 0:1],
                                         exs[:, DS * pr:DS * pr + DS, :],
                                         start=(pr == 0), stop=(pr == NPAIR - 1),
                                         perf_mode=PM)

                    for kc in range(NKT):
                        lg = psum.tile([P, W5], F32, tag="lg", bufs=3, name="lg")
                        for fp in range(NFC // DS):
                            nc.tensor.matmul(lg, KT[:, DS * fp:DS * fp + DS, kc * P:(kc + 1) * P],
                                             QT[:, DS * fp:DS * fp + DS, sl],
                                             start=(fp == 0), stop=(fp == NFC // DS - 1),
                                             perf_mode=PM)
                        nc.scalar.activation(exs[:, kc, :], lg, AF.Exp,
                                             bias=ebias[:, 0:1], scale=SM_SCALE)
                        # z group (kc//DS - 1) one group late so exp is done
                        if kc % DS == DS - 1 and kc >= DS:
                            z_acc(kc // DS - 1)

                    # attn@V, one 128-wide q chunk at a time; first chunk's
                    # matmuls hide the final z group + Z->rz transpose latency
                    if sub == "noattn":
                        continue
                    rzc = bp.tile([P, NST], F32, tag="rzc", name="rzc")
                    if sub == "noz":
                        nc.vector.memset(rzc, 1.0)
                    for qq in range(NST):
                        qsl = slice(qq * P, (qq + 1) * P)
                        aos = [psum.tile([P, W5], F32, tag="ao", name=f"ao{dh}")
                               for dh in range(NDH)]
                        for pr in range(NPAIR):
                            for dh in range(NDH):
                                nc.tensor.matmul(aos[dh], exs[:, DS * pr:DS * pr + DS, qsl],
                                                 V[:, DS * pr:DS * pr + DS, dh * W5:(dh + 1) * W5],
                                                 start=(pr == 0), stop=(pr == NPAIR - 1),
                                                 perf_mode=PM)
                            if qq == 0 and pr == 0 and sub != "noz":
                                z_acc(NPAIR - 1)
                                # Z row -> [128,4] per-partition scalars via a
                                # DRAM bounce (SBUF APs can't express the
                                # transposing view) -> 1/(Z+eps)
                                z2 = bp.tile([P, W5], F32, tag="z2", bufs=1, name="z2")
                                nc.vector.tensor_scalar_add(z2[0:1, :], zps[0:1, :], EPS)
                                zdr = dram.tile([1, W5], F32, tag="zdr")
                                nc.sync.dma_start(out=zdr, in_=z2[0:1, :])
                                zc = bp.tile([P, NST], F32, tag="zc", name="zc")
                                nc.sync.dma_start(
                                    out=zc,
                                    in_=zdr.rearrange("a (q p) -> (a p) q", p=P))
                                nc.vector.reciprocal(rzc, zc)
                        for dh in range(NDH):
                            ob = bp.tile([P, W5], F32, tag="osb", name="ob")
                            nc.vector.scalar_tensor_tensor(
                                ob, aos[dh], rzc[:, qq:qq + 1],
                                xrs[:, qq, dh * W5:(dh + 1) * W5],
                                op0=ALU.mult, op1=ALU.add)
                            nc.sync.dma_start(
                                out=out[b, st * W5 + qq * P:st * W5 + (qq + 1) * P,
                                        dh * W5:(dh + 1) * W5],
                                in_=ob)

    nc.compile()
    return nc


def _build_pe(KBENCH="pairev"):
    """Paired-eviction design: every ACT/DVE psum eviction covers TWO psum
    banks ([128,1024]) and output DMAs write full-D rows.  PSUM: pp pairs
    (2x2 banks) + lg (3) + z (1) = 8 banks.

    Q/K biases are folded per-partition via stride-0 broadcast APs; V bias
    via the bvb row tile. Exp stays per-bank (lg tiles are per-k-chunk).

    KBENCH sub-variants: "..._proj" stops after stage A; "..._gpsv" moves the
    V-cast eviction to the GPSIMD (Pool) engine to unload DVE.
    """
    import concourse.bass as bass
    SUB = KBENCH.split("_")[-1] if "_" in KBENCH else ""
    BF16 = _is_bf16(KBENCH)
    MDT = mybir.dt.bfloat16 if BF16 else F8
    DS = 1 if BF16 else 2
    PM = None if BF16 else DR
    nb = 2 if BF16 else 1
    NPAIR = NKT // DS

    nc = bacc.Bacc("TRN2", target_bir_lowering=False, debug=False)

    u8 = mybir.dt.uint8
    x8T = nc.dram_tensor("x8T", [BPC, NST, P, NDC * W5 * nb], u8,
                         kind="ExternalInput").ap().bitcast(MDT)
    y8T = nc.dram_tensor("y8T", [BPC, NST, P, NDC * W5 * nb], u8,
                         kind="ExternalInput").ap().bitcast(MDT)
    xr = nc.dram_tensor("xr", [BPC, S, D], F32, kind="ExternalInput").ap()
    w8q = nc.dram_tensor("w8q", [P, NDC * D * nb], u8, kind="ExternalInput").ap().bitcast(MDT)
    w8k = nc.dram_tensor("w8k", [P, NDC * D * nb], u8, kind="ExternalInput").ap().bitcast(MDT)
    w8v = nc.dram_tensor("w8v", [P, NDC * D * nb], u8, kind="ExternalInput").ap().bitcast(MDT)
    bq = nc.dram_tensor("bq", [P, NFC], F32, kind="ExternalInput").ap()
    bk = nc.dram_tensor("bk", [P, NFC], F32, kind="ExternalInput").ap()
    bv = nc.dram_tensor("bv", [D], F32, kind="ExternalInput").ap()
    out = nc.dram_tensor("out", [BPC, S, D], F32, kind="ExternalOutput").ap()

    RSC = float(1.0 / WSCALE)

    def bc2(t, fc):
        # [P, 2, W5] stride-0 view of bias columns fc, fc+1 of a [P, NFC] tile
        a = t[:, fc:fc + 2]
        return bass.AP(tensor=a.tensor, offset=a.offset,
                       ap=list(a.ap) + [[0, W5]])

    with tile.TileContext(nc) as tc, ExitStack() as ctx:
        const = ctx.enter_context(tc.tile_pool(name="const", bufs=1))
        kvp = ctx.enter_context(tc.tile_pool(name="kvp", bufs=1))
        expp = ctx.enter_context(tc.tile_pool(name="expp", bufs=1))
        psum = ctx.enter_context(tc.tile_pool(name="psum", bufs=2, space="PSUM"))
        dram = ctx.enter_context(tc.tile_pool(name="dram", bufs=2, space="DRAM"))

        onesf = const.tile([P, 2, 16], F32)
        nc.vector.memset(onesf, 1.0)
        ones8 = const.tile([P, 2, 16], MDT)
        nc.vector.tensor_copy(ones8, onesf)
        bqs = const.tile([P, NFC], F32)
        nc.gpsimd.dma_start(out=bqs, in_=bq)
        bks = const.tile([P, NFC], F32)
        nc.gpsimd.dma_start(out=bks, in_=bk)
        bvb = const.tile([P, D], F32)
        bv1 = bv.rearrange("(a d) -> a d", a=1)
        bv_bcast = bass.AP(tensor=bv1.tensor, offset=bv1.offset,
                           ap=[[0, P]] + list(bv1.ap[1:]))
        nc.gpsimd.dma_start(out=bvb, in_=bv_bcast)
        ebias = const.tile([P, 1], F32)
        nc.vector.memset(ebias, EXP_BIAS)

        wq = const.tile([P, NDC, D], MDT, name="wq")
        nc.sync.dma_start(out=wq, in_=w8q.rearrange("p (dc f) -> p dc f", dc=NDC))
        wk = const.tile([P, NDC, D], MDT, name="wk")
        nc.sync.dma_start(out=wk, in_=w8k.rearrange("p (dc f) -> p dc f", dc=NDC))
        wv = const.tile([P, NDC, D], MDT, name="wv")
        nc.sync.dma_start(out=wv, in_=w8v.rearrange("p (dc f) -> p dc f", dc=NDC))

        for b in range(BPC):
            QT = kvp.tile([P, NFC, S], MDT, tag="QT")
            KT = kvp.tile([P, NFC, S], MDT, tag="KT")
            V = kvp.tile([P, NKT, D], MDT, tag="V")

            # ---------------- stage A ----------------
            with tc.tile_pool(name=f"stA_{b}", bufs=2) as strips:
                for st in range(NST):
                    sl = slice(st * W5, (st + 1) * W5)
                    xq = strips.tile([P, NDC, W5], MDT, tag="xq", name="xq", bufs=2)
                    nc.sync.dma_start(out=xq, in_=x8T[b, st].rearrange("p (dc s) -> p dc s", dc=NDC))
                    yq = strips.tile([P, NDC, W5], MDT, tag="yq", name="yq", bufs=2)
                    nc.sync.dma_start(out=yq, in_=y8T[b, st].rearrange("p (dc s) -> p dc s", dc=NDC))

                    # Q then K: evict two fc banks per instr
                    for w_, dst, bt in ((wq, QT, bqs), (wk, KT, bks)):
                        for fc2 in range(NFC // 2):
                            pp = psum.tile([P, 2, W5], F32, tag="pp", bufs=2, name="pp")
                            for j in range(2):
                                fc = 2 * fc2 + j
                                for dp in range(NDC // DS):
                                    nc.tensor.matmul(
                                        pp[:, j, :],
                                        w_[:, DS * dp:DS * dp + DS, fc * P:(fc + 1) * P],
                                        xq[:, DS * dp:DS * dp + DS, :] if w_ is wq
                                        else yq[:, DS * dp:DS * dp + DS, :],
                                        start=(dp == 0), stop=(dp == NDC // DS - 1),
                                        perf_mode=PM)
                            # stt with stride-0 bias broadcast (honest bias);
                            # ACT can't take a pair bias (strict [P,1]), so
                            # Q/K go to DVE (or K to GPSIMD in gpsall).
                            keng = nc.gpsimd if (SUB == "gpsall" and w_ is wk) \
                                else nc.vector
                            keng.scalar_tensor_tensor(
                                dst[:, 2 * fc2:2 * fc2 + 2, sl], pp, RSC,
                                bc2(bt, 2 * fc2), op0=ALU.mult, op1=ALU.add)

                    # V: evict both dh halves of a seq chunk per instr
                    for ks in range(NST):
                        kt = st * NST + ks
                        pp = psum.tile([P, 2, W5], F32, tag="pp", bufs=2, name="ppv")
                        for dh in range(NDH):
                            for dp in range(NDC // DS):
                                nc.tensor.matmul(pp[:, dh, :],
                                                 yq[:, DS * dp:DS * dp + DS, ks * P:(ks + 1) * P],
                                                 wv[:, DS * dp:DS * dp + DS, dh * W5:(dh + 1) * W5],
                                                 start=(dp == 0), stop=(dp == NDC // DS - 1),
                                                 perf_mode=PM)
                        veng = nc.gpsimd if SUB in ("gpsv", "gpsall") else nc.vector
                        veng.scalar_tensor_tensor(
                            V[:, kt, :].rearrange("p (t w) -> p t w", t=2), pp, RSC,
                            bvb.rearrange("p (t w) -> p t w", t=2),
                            op0=ALU.mult, op1=ALU.add)

            # ---------------- stage B ----------------
            if SUB == "proj":
                continue
            with tc.tile_pool(name=f"stB_{b}", bufs=4) as bp:
                for st in range(NST):
                    sl = slice(st * W5, (st + 1) * W5)
                    exs = expp.tile([P, NKT, W5], MDT, tag="exT", bufs=2)
                    xrs = bp.tile([P, NST, D], F32, tag="xrs", bufs=2, name="xrs")
                    nc.sync.dma_start(
                        out=xrs, in_=xr[b, sl, :].rearrange("(c p) d -> p c d", p=P))
                    zps = psum.tile([P, W5], F32, tag="z", bufs=1, name="zps")

                    def z_acc(pr):
                        nc.tensor.matmul(zps[0:1, :], ones8[:, 0:DS, 0:1],
                                         exs[:, DS * pr:DS * pr + DS, :],
                                         start=(pr == 0), stop=(pr == NPAIR - 1),
                                         perf_mode=PM)

                    for kc in range(NKT):
                        lg = psum.tile([P, W5], F32, tag="lg", bufs=3, name="lg")
                        for fp in range(NFC // DS):
                            nc.tensor.matmul(lg, KT[:, DS * fp:DS * fp + DS, kc * P:(kc + 1) * P],
                                             QT[:, DS * fp:DS * fp + DS, sl],
                                             start=(fp == 0), stop=(fp == NFC // DS - 1),
                                             perf_mode=PM)
                        nc.scalar.activation(exs[:, kc, :], lg, AF.Exp,
                                             bias=ebias[:, 0:1], scale=SM_SCALE)
                        if kc % DS == DS - 1 and kc >= DS:
                            z_acc(kc // DS - 1)

                    rzc = bp.tile([P, NST], F32, tag="rzc", name="rzc")
                    for qq in range(NST):
                        qsl = slice(qq * P, (qq + 1) * P)
                        aos = psum.tile([P, 2, W5], F32, tag="pp", bufs=2, name="aos")
                        for pr in range(NPAIR):
                            for dh in range(NDH):
                                nc.tensor.matmul(aos[:, dh, :], exs[:, DS * pr:DS * pr + DS, qsl],
                                                 V[:, DS * pr:DS * pr + DS, dh * W5:(dh + 1) * W5],
                                                 start=(pr == 0), stop=(pr == NPAIR - 1),
                                                 perf_mode=PM)
                            if qq == 0 and pr == 0:
                                z_acc(NPAIR - 1)
                                z2 = bp.tile([P, W5], F32, tag="z2", bufs=1, name="z2")
                                nc.vector.tensor_scalar_add(z2[0:1, :], zps[0:1, :], EPS)
                                zdr = dram.tile([1, W5], F32, tag="zdr")
                                # Pool queue: keeps the ~512 4B-descriptor
                                # gather off the SP queues carrying out-writes
                                zq = nc.gpsimd if SUB in ("gpsall", "zq") else nc.sync
                                zq.dma_start(out=zdr, in_=z2[0:1, :])
                                zc = bp.tile([P, NST], F32, tag="zc", name="zc")
                                zq.dma_start(
                                    out=zc,
                                    in_=zdr.rearrange("a (q p) -> (a p) q", p=P))
                                nc.vector.reciprocal(rzc, zc)
                        ob = bp.tile([P, D], F32, tag="osb", name="ob")
                        nc.vector.scalar_tensor_tensor(
                            ob.rearrange("p (t w) -> p t w", t=2), aos, rzc[:, qq:qq + 1],
                            xrs[:, qq, :].rearrange("p (t w) -> p t w", t=2),
                            op0=ALU.mult, op1=ALU.add)
                        nc.sync.dma_start(
                            out=out[b, st * W5 + qq * P:st * W5 + (qq + 1) * P, :],
                            in_=ob)

    nc.compile()
    return nc


_NC_CACHE = {}


def _get_nc(KBENCH=None):
    key = KBENCH_DEFAULT if KBENCH is None else KBENCH
    if key not in _NC_CACHE:
        if key in ("full", "pairev") or "pairev" in key:
            _NC_CACHE[key] = _build_pe(key)
        else:
            _NC_CACHE[key] = _build(key)
    return _NC_CACHE[key]


def _tile_strips(a, mdt):
    """[B, S, D] fp32 -> [B, NST, P, NDC*W5*nb] u8: strip-tile layout where
    tile[p, dc, s'] = a[b, st*W5+s', dc*P+p] (feature-major per strip)."""
    B_ = a.shape[0]
    t = a.astype(mdt)                                    # [B, S, D]
    t = t.reshape(B_, NST, W5, NDC, P)                   # S=(st,s'), D=(dc,p)
    t = np.ascontiguousarray(t.transpose(0, 1, 4, 3, 2))  # [B, st, p, dc, s']
    return t.reshape(B_, NST, P, -1).view(np.uint8).reshape(B_, NST, P, -1)


def _tile_w(w, mdt):
    """[D, D] fp32 -> [P, NDC*D*nb] u8 with tile[p, dc, f] = w[dc*P+p, f]."""
    t = (np.asarray(w, dtype=np.float32) * WSCALE).astype(mdt)
    t = np.ascontiguousarray(t.reshape(NDC, P, D).transpose(1, 0, 2))
    return t.reshape(P, -1).view(np.uint8).reshape(P, -1)


def _make_in_maps(x, y, Wq, bq, Wk, bk, Wv, bv, KBENCH=None):
    kb = KBENCH_DEFAULT if KBENCH is None else KBENCH
    mdt = np.dtype(mybir.dt.np(mybir.dt.bfloat16 if _is_bf16(kb) else F8))
    x = np.asarray(x, dtype=np.float32)
    y = np.asarray(y, dtype=np.float32)
    x8T = _tile_strips(x, mdt)
    y8T = _tile_strips(y, mdt)
    w8q = _tile_w(Wq, mdt)
    w8k = _tile_w(Wk, mdt)
    w8v = _tile_w(Wv, mdt)
    bq = np.ascontiguousarray(np.asarray(bq, dtype=np.float32).reshape(NFC, P).T)
    bk = np.ascontiguousarray(np.asarray(bk, dtype=np.float32).reshape(NFC, P).T)
    bv = np.ascontiguousarray(np.asarray(bv, dtype=np.float32))
    in_maps = []
    for c in range(NCORES):
        sl = slice(c * BPC, (c + 1) * BPC)
        in_maps.append({
            "x8T": np.ascontiguousarray(x8T[sl]),
            "y8T": np.ascontiguousarray(y8T[sl]),
            "xr": np.ascontiguousarray(x[sl]),
            "w8q": w8q, "w8k": w8k, "w8v": w8v,
            "bq": bq, "bk": bk, "bv": bv,
        })
    return in_maps


def kernel(x, y, Wq, bq, Wk, bk, Wv, bv):
    nc = _get_nc()
    in_maps = _make_in_maps(x, y, Wq, bq, Wk, bk, Wv, bv)
    res = run_bass_kernel_spmd(nc, in_maps, core_ids=list(range(NCORES)))
    return np.concatenate([r["out"] for r in res.results], axis=0)
